# revision 2
# baseline (speedup 1.0000x reference)
"""GCNConv-with-constraint kernel for 8 Trainium2 NeuronCores — v2.

Design: the target environment charges a large fixed cost per compute-engine
instruction, so the kernel minimizes instruction count.

out = D^-1/2 (A+I) D^-1/2 x Wn^T + b, with Wn = column-renormed W.

Host prep (structure + diagonal scalings only):
  - xs = x * dinv[:,None] in fp16 -> gather table T with two zero pad rows
    (row 0 for the lo half, row 32768 for the hi half; int16 gather indices
    address 32768 rows per half).
  - edges sharded by dst (6250 dsts/core); per dst, srcs split into lo/hi
    lists; (dst) assigned to a joint tier (KL, KH) sized to fit both lists;
    each dst gets one fixed-KL lo segment and one fixed-KH hi segment,
    padded with zero-row indices. Tier sizes are global (max over cores).
  - WnT (renormed, transposed) fp16.

Device per core (few, large instructions):
  - dma_gather(transpose=True) pulls segments as columns of msgsT [c, slot]
    in ~8192-idx calls into a [128, RCHUNK] fp16 buffer.
  - one tensor_reduce per superchunk sums each dst's K-segment:
    [128, D, K] -> [128, D] fp32, written into agg strips (lo and hi).
  - merge+cast: aggc = f16(agg_lo + agg_hi) in one DVE op.
  - epilogue: outT[:, j] = WnT.T @ aggc columns, 512-wide matmuls into one
    8-bank PSUM strip, 2 PSUM->SBUF copies, 1 DMA out.

Host post: out[dst] = outT.T * dinv[dst] + b (diagonal scale + bias),
reassembled across cores/tiers via dstmap.
"""

import math
import os
from contextlib import ExitStack

import numpy as np

import concourse.bass as bass
import concourse.tile as tile
from concourse import bacc, mybir
from concourse.bass_utils import run_bass_kernel_spmd

N_CORES = 8
C = 128
P = 128
LOCUT = 32767  # srcs < LOCUT -> lo table rows 1..32767; else hi rows 1..
HIBASE = 32768  # hi table base row (T[HIBASE] = 0 pad row)
GCHUNK = 8192  # max idxs per dma_gather instruction (ring-safe)
RCHUNK = 24576  # slots per gather buffer / reduce superchunk
TIERS = [(24, 16), (32, 24), (48, 40)]  # (KL, KH) joint tiers; last catches all
MMW = 512  # epilogue matmul width
PSW = 4096  # psum strip width (8 banks x 512 f32)

f16 = mybir.dt.float16
f32 = mybir.dt.float32
i16 = mybir.dt.int16

LAST_RESULTS = None


def _align_up(x, m):
    return (x + m - 1) // m * m


def _prep(x, edge_index, W, b):
    N = x.shape[0]
    assert N % N_CORES == 0
    npc = N // N_CORES

    src = np.asarray(edge_index[0], dtype=np.int64)
    dst = np.asarray(edge_index[1], dtype=np.int64)

    deg = np.bincount(dst, minlength=N).astype(np.float64) + 1.0
    dinv = 1.0 / np.sqrt(deg)

    # gather table with per-half zero pad rows
    xs = (np.asarray(x, np.float64) * dinv[:, None]).astype(np.float16)
    nhi = N - LOCUT
    T = np.zeros((HIBASE + 1 + nhi, C), np.float16)
    T[1 : 1 + LOCUT] = xs[:LOCUT]
    T[HIBASE + 1 :] = xs[LOCUT:]

    # self loops
    ar = np.arange(N, dtype=np.int64)
    src_all = np.concatenate([src, ar])
    dst_all = np.concatenate([dst, ar])

    shard = dst_all // npc
    dst_loc = dst_all - shard * npc
    ishi = src_all >= LOCUT

    # per-core per-dst lo/hi source lists (as sorted flat arrays + counts)
    # key = dst_loc*2 + ishi
    percore = []
    ntier = len(TIERS)
    cnt_gc = np.zeros((N_CORES, ntier), np.int64)  # dsts per (core, tier)
    for s in range(N_CORES):
        m = shard == s
        sl_src = src_all[m]
        sl_hi = ishi[m]
        sl_dl = dst_loc[m]
        key = sl_dl * 2 + sl_hi
        order = np.argsort(key, kind="stable")
        ssrc = sl_src[order]
        cnt = np.bincount(key, minlength=npc * 2)
        L, H = cnt[0::2], cnt[1::2]
        kl_max, kh_max = TIERS[-1]
        assert L.max() <= kl_max and H.max() <= kh_max, (L.max(), H.max())
        tier = np.full(npc, ntier - 1, np.int64)
        for t in range(ntier - 2, -1, -1):
            kl, kh = TIERS[t]
            tier[(L <= kl) & (H <= kh)] = t
        for t in range(ntier):
            cnt_gc[s, t] = int((tier == t).sum())
        starts = np.concatenate([[0], np.cumsum(cnt)])
        percore.append(dict(ssrc=ssrc, starts=starts, L=L, H=H, tier=tier))

    # common group sizes (padded, 16-aligned)
    D_g = [_align_up(max(1, int(cnt_gc[:, t].max())), 16) for t in range(ntier)]
    Dtot = sum(D_g)
    Dpad = _align_up(Dtot, MMW)

    # stream slot layout (common): lo stream = groups in order, D_g*KL slots
    lo_off, hi_off = [], []
    o = 0
    for t in range(ntier):
        lo_off.append(o)
        o += D_g[t] * TIERS[t][0]
    lo_slots = o
    o = 0
    for t in range(ntier):
        hi_off.append(o)
        o += D_g[t] * TIERS[t][1]
    hi_slots = o

    # superchunks + reduce runs (common structure)
    def make_sched(offs, slots, kidx):
        # returns list of superchunks: (slot_off, n_slots, [(d_off, D, K, off_in_sc)])
        # and gather calls per sc: [(idx_off, n)]
        scs = []
        for t in range(ntier):
            K = TIERS[t][kidx]
            doff = sum(D_g[:t])
            # subdivide group into scs of <= RCHUNK slots, D-sub multiple of 16
            dmax = (RCHUNK // (16 * K)) * 16
            d0 = 0
            while d0 < D_g[t]:
                dn = min(dmax, D_g[t] - d0)
                soff = offs[t] + d0 * K
                ns = dn * K
                scs.append((soff, ns, doff + d0, dn, K))
                d0 += dn
        return scs

    lo_scs = make_sched(lo_off, lo_slots, 0)
    hi_scs = make_sched(hi_off, hi_slots, 1)

    # per-core idx arrays + dstmap
    in_maps = []
    wf = np.asarray(W, np.float64)
    norms = np.sqrt((wf**2).sum(axis=0))
    scale = np.minimum(1.0, 1.0 / np.maximum(norms, 1e-30))
    WnT = np.ascontiguousarray((wf * scale[None, :]).T.astype(np.float16))

    for s in range(N_CORES):
        pc = percore[s]
        ssrc, starts, tier = pc["ssrc"], pc["starts"], pc["tier"]
        idx_lo = np.zeros(lo_slots, np.int16)
        idx_hi = np.zeros(hi_slots, np.int16)
        dstmap = np.full(Dtot, -1, np.int64)
        for t in range(ntier):
            kl, kh = TIERS[t]
            doff = sum(D_g[:t])
            ds = np.where(tier == t)[0]
            for j, d in enumerate(ds):
                dstmap[doff + j] = d
                a0, a1 = starts[2 * d], starts[2 * d + 1]
                b0, b1 = starts[2 * d + 1], starts[2 * d + 2]
                ls = ssrc[a0:a1]
                hs = ssrc[b0:b1]
                p0 = lo_off[t] + (doff - sum(D_g[:t]) + j) * kl
                idx_lo[p0 : p0 + len(ls)] = (ls + 1).astype(np.int16)
                p1 = hi_off[t] + j * kh
                idx_hi[p1 : p1 + len(hs)] = (hs - LOCUT + 1).astype(np.int16)
        wlo = np.ascontiguousarray(np.tile(idx_lo.reshape(-1, 16).T, (8, 1)))
        whi = np.ascontiguousarray(np.tile(idx_hi.reshape(-1, 16).T, (8, 1)))
        in_maps.append(
            {"tt": T, "idxlo": wlo, "idxhi": whi, "wnt": WnT, "_dstmap": dstmap}
        )

    st = dict(
        N=N, npc=npc, Dtot=Dtot, Dpad=Dpad,
        lo_slots=lo_slots, hi_slots=hi_slots,
        lo_scs=lo_scs, hi_scs=hi_scs,
        trows=T.shape[0],
        dinv=dinv, bias=np.asarray(b, np.float64),
    )
    return in_maps, st


def _build_program(st, repeat=1):
    Dtot, Dpad = st["Dtot"], st["Dpad"]
    nc = bacc.Bacc("TRN2", target_bir_lowering=False, debug=False,
                   num_devices=N_CORES)

    tt = nc.dram_tensor("tt", [st["trows"], C], f16, kind="ExternalInput").ap()
    idxlo = nc.dram_tensor("idxlo", [P, st["lo_slots"] // 16], i16,
                           kind="ExternalInput").ap()
    idxhi = nc.dram_tensor("idxhi", [P, st["hi_slots"] // 16], i16,
                           kind="ExternalInput").ap()
    wnt = nc.dram_tensor("wnt", [C, C], f16, kind="ExternalInput").ap()
    outd = nc.dram_tensor("outd", [C, Dtot], f32, kind="ExternalOutput").ap()

    Copy = mybir.ActivationFunctionType.Copy
    Op = mybir.AluOpType
    X = mybir.AxisListType.X

    tthi = tt[HIBASE:, :]

    with tile.TileContext(nc) as tc, ExitStack() as ctx:
        cpool = ctx.enter_context(tc.tile_pool(name="const", bufs=1))
        wnt_sb = cpool.tile([C, C], f16, tag="wnt")
        nc.sync.dma_start(wnt_sb[:], wnt[:])

        apool = ctx.enter_context(tc.tile_pool(name="aggs", bufs=1))
        agg_lo = apool.tile([P, Dtot], f32, tag="agglo")
        agg_hi = apool.tile([P, Dtot], f32, tag="agghi")
        aggc = apool.tile([P, Dpad], f16, tag="aggc")
        outsb = apool.tile([C, Dtot], f32, tag="outsb")
        nc.vector.memset(aggc[:], 0.0)

        gpool = ctx.enter_context(tc.tile_pool(name="gb", bufs=2))
        ipool = ctx.enter_context(tc.tile_pool(name="idx", bufs=3))
        pspool = ctx.enter_context(tc.tile_pool(name="ps", bufs=1, space="PSUM"))

        for rep in range(repeat):
            for half, scs, idxs in (("lo", st["lo_scs"], idxlo),
                                    ("hi", st["hi_scs"], idxhi)):
                agg = agg_lo if half == "lo" else agg_hi
                src_ap = tt[:] if half == "lo" else tthi
                for si, (soff, ns, d0, dn, K) in enumerate(scs):
                    gb = gpool.tile([P, 1, RCHUNK], f16, tag="gb",
                                    name=f"gb_{rep}_{half}_{si}")
                    o = 0
                    while o < ns:
                        cn = min(GCHUNK, ns - o)
                        it = ipool.tile([P, cn // 16], i16, tag="it",
                                        name=f"it_{rep}_{half}_{si}_{o}")
                        c0 = (soff + o) // 16
                        nc.sync.dma_start(it[:], idxs[:, c0 : c0 + cn // 16])
                        nc.gpsimd.dma_gather(
                            out_ap=gb[:, :, o : o + cn],
                            in_ap=src_ap,
                            idxs_ap=it[:],
                            num_idxs=cn,
                            num_idxs_reg=cn,
                            elem_size=C,
                            transpose=True,
                            single_packet=False,
                        )
                        o += cn
                    nc.vector.tensor_reduce(
                        out=agg[:, d0 : d0 + dn],
                        in_=gb[:, :, :ns].rearrange("p o (d k) -> p (o d) k", k=K),
                        axis=X, op=Op.add,
                    )
            # merge + cast to f16 in one op
            nc.vector.tensor_tensor(out=aggc[:, :Dtot], in0=agg_lo[:],
                                    in1=agg_hi[:], op=Op.add)
            # epilogue: outT = WnT.T @ aggc
            j = 0
            while j < Dtot:
                bw = min(PSW, Dpad - j)
                ps = pspool.tile([C, PSW], f32, tag="ps", name=f"ps_{rep}_{j}")
                jj = 0
                while jj < bw:
                    nc.tensor.matmul(
                        ps[:, jj : jj + MMW],
                        lhsT=wnt_sb[:],
                        rhs=aggc[:, j + jj : j + jj + MMW],
                        start=True, stop=True,
                    )
                    jj += MMW
                cw = min(bw, Dtot - j)
                nc.scalar.activation(outsb[:, j : j + cw], ps[:, :cw], Copy)
                j += bw
            nc.sync.dma_start(outd[:], outsb[:])

    nc.compile()
    return nc


def kernel(x, edge_index, W, b):
    global LAST_RESULTS
    x = np.asarray(x)
    N = x.shape[0]
    assert x.shape[1] == C and W.shape == (C, C)

    in_maps, st = _prep(x, edge_index, W, b)
    nc = _build_program(st)

    os.environ.setdefault("BASS_NEVER_TRACE", "1")
    dev_maps = [{k: v for k, v in m.items() if not k.startswith("_")}
                for m in in_maps]
    res = run_bass_kernel_spmd(nc, dev_maps, list(range(N_CORES)))
    LAST_RESULTS = res

    npc = st["npc"]
    out = np.zeros((N, C), np.float64)
    for s in range(N_CORES):
        outd = res.results[s]["outd"]  # [C, Dtot] f32
        dstmap = in_maps[s]["_dstmap"]
        valid = dstmap >= 0
        out[s * npc + dstmap[valid]] = outd[:, valid].T
    out = out * st["dinv"][:, None] + st["bias"][None, :]
    return np.ascontiguousarray(out.astype(np.float32))
